# revision 18
# baseline (speedup 1.0000x reference)
"""Trainium2 Bass kernel for the neural-ODE Euler integration problem.

Model (per batch element b, nsteps sequential steps):
    h~ = elu(s) + 1 computed exactly as  h~ = max(s+1, min(exp(s), 1))
    x <- x + dt * (h3~ @ W4 + b4')        b4' = b4 - colsum(W4)

Structure: 8 cores x 2 independent chains of 64 trajectories each
(G=2 hides the serial dependency chain; N=64 matmuls).  Per chain the
"+1" is pre-accumulated into PSUM via bias matmuls so each layer is:
    exp (Act, bias=-1) -> min (GPSIMD) -> max-combine (DVE TT/STT from
    PSUM) -> K=128 fp16 stream matmuls.
W14 fusion: a1_{j} = Ux@x~_j + W14@h3d_{j-1} with W14 = W4@Ux and
h3d = dt*h3~ (dt folded exactly: dt*max(a,m) = max(dt*a, dt*m)), so the
inter-step critical path skips euler/cast/stage; the real x update runs
with 2 steps of slack.  x~_j = x_{j-1} + dt_{j-1}*b4'.
z_eff (event switch) is computed on host, packed fp16 at partition
bases {0,32,64,96} (engine APs require 32-aligned partition starts).
"""

import numpy as np
import sys

if '/opt/trn_rl_repo' not in sys.path:
    sys.path.insert(0, '/opt/trn_rl_repo')

import concourse.bass as bass
import concourse.bacc as bacc
import concourse.mybir as mybir
from concourse.tile import TileContext
from concourse import bass_utils

F32 = mybir.dt.float32
F16 = mybir.dt.float16
AF = mybir.ActivationFunctionType
OP = mybir.AluOpType

B, T, XD, ZD, HID = 1024, 1000, 8, 8, 256
NCORES = 8
PCB = B // NCORES         # trajectories per core = 128
G = 2                     # independent chains per core
PB = PCB // G             # trajectories per chain = 64
RG = 16                   # ring group: steps per output DMA flush
NSTEPS = T - 1

LAST_RESULTS = None       # set by kernel(): BassKernelResults


def _build(nsteps):
    """Build + compile the Bass program (same program for all 8 cores)."""
    nc = bacc.Bacc("TRN2", target_bir_lowering=False, debug=False,
                   num_devices=NCORES)

    ns = nsteps
    nzb = (ns + 3) // 4            # zq4 column blocks (4 steps each)
    nrg = (ns + RG - 1) // RG      # ring groups
    nring = nrg * RG + 1

    d = {}
    def din(name, shape, dt):
        d[name] = nc.dram_tensor(name, shape, dt, kind="ExternalInput").ap()

    # per-chain tensors get a _0/_1 suffix
    for c in range(G):
        din(f"zq4_{c}", [128, nzb * PB], F16)   # z_eff packed 4 steps/block
        din(f"st0_{c}", [40, PB], F16)          # stage for step 0
        din(f"st1_{c}", [40, PB], F16)          # stage for step 1
        din(f"x0f_{c}", [8, PB], F32)           # x0 fp32
    din("vux",  [40, 256], F16)     # rows: Ux(0-7) V(8-23) b1+1 hilo(24-25) Uz(32-39)
    din("w2p",  [128, 512], F16)    # W2[kc*128+p, h*128+m] at col (kc*2+h)*128+m
    din("w3p",  [128, 512], F16)
    din("w14p", [128, 512], F16)    # W14 = W4 @ Ux, same packing
    din("w4p",  [128, 16], F16)     # W4[kc*128+p, m] at col kc*8+m
    din("b2e",  [4, 128], F16)      # hilo(b2'+1) half0 rows 0-1, half1 rows 2-3
    din("b3e",  [4, 128], F16)
    din("db4t", [2, 8 * ns], F16)   # hilo(dt_i * b4') lhsT slices per step
    din("ones2", [2, G * PB], F16)  # all-ones rhs for the db4 matmul
    din("bias_ones", [4, 2 * G * PB], F16)  # half-selector rows, chain-tiled
    din("dtc",  [128, ns], F32)     # dt_i replicated down partitions (scalar AP)
    din("db4s", [8, ns], F32)       # dt_i * b4' columns (x~ bias)
    xout_d = nc.dram_tensor("xout_d", [8, nring, PCB], F32,
                            kind="ExternalOutput").ap()

    with TileContext(nc) as tc:
        with tc.tile_pool(name="const", bufs=1) as cpool, \
             tc.tile_pool(name="work", bufs=3) as wpool, \
             tc.tile_pool(name="psum", bufs=2, space="PSUM") as ppool:

            sb = {}
            for name in d:
                shape = [int(s) for s in d[name].shape]
                sb[name] = cpool.tile(shape, d[name].dtype, name=name, tag=name)
                nc.sync.dma_start(out=sb[name][:], in_=d[name])

            negones = cpool.tile([128, 1], F32, name="negones", tag="negones")
            nc.vector.memset(negones[:], -1.0)

            # rotating stage tiles (4 per chain; writes happen 2 steps ahead)
            stages = [[None] * 4 for _ in range(G)]
            for c in range(G):
                for k in range(4):
                    st = wpool.tile([40, PB], F16, name=f"st{c}_{k}",
                                    tag=f"st{c}_{k}", bufs=1)
                    # st0 carries the static init/ones rows (8-25); for
                    # k>=2 the per-step rows (0-7, 32-39) are overwritten
                    # in-loop before first read.
                    nc.sync.dma_start(
                        out=st[:], in_=d[f"st1_{c}" if k == 1 else f"st0_{c}"])
                    stages[c][k] = st

            # PSUM tiles are shared across chains (chain c owns columns
            # c*2PB..(c+1)*2PB) so the pool fits in 8 banks.
            a1t = dict()                        # step j -> a1 PSUM tile
            xprev = [None] * G                  # AP of x_i (f32)
            rings = [dict() for _ in range(G)]

            def chs(tile, c):
                return tile[:, c * 2 * PB:(c + 1) * 2 * PB]

            def open_a1(j):
                a1 = ppool.tile([128, 2 * G * PB], F32, name="a1",
                                tag="a1", bufs=2)
                a1t[j] = a1
                return a1

            # NOTE PSUM start=True resets has_written state bank-wide: each
            # (chain, half) region must fully accumulate before the next
            # region's start executes, or use a single start per bank.
            def a1_tail_mms(c, j, h3d):
                """Per region: W14 kc0 (start) -> kc1 -> stage mm (stop)."""
                a1 = a1t[j]
                st = stages[c][j % 4]
                for h in range(2):
                    reg = chs(a1, c)[:, h * PB:(h + 1) * PB]
                    for kc in range(2):
                        lh = sb["w14p"][:, (kc * 2 + h) * 128:(kc * 2 + h + 1) * 128]
                        nc.tensor.matmul(reg, lhsT=lh,
                                         rhs=h3d[:, kc * PB:(kc + 1) * PB],
                                         start=(kc == 0), stop=False,
                                         skip_group_check=True)
                    nc.tensor.matmul(reg,
                                     lhsT=sb["vux"][:, h * 128:(h + 1) * 128],
                                     rhs=st[:], start=False, stop=True,
                                     skip_group_check=True)

            def a1_first_mms():
                """a1_0: stage matmuls only, each region a complete group."""
                a1 = open_a1(0)
                for c in range(G):
                    st = stages[c][0]
                    for h in range(2):
                        nc.tensor.matmul(
                            chs(a1, c)[:, h * PB:(h + 1) * PB],
                            lhsT=sb["vux"][:, h * 128:(h + 1) * 128],
                            rhs=st[:], start=True, stop=True,
                            skip_group_check=True)

            def layer(c, i, l, a_cur, w_next, h3d_mode):
                """exp -> min -> max-combine; returns h tile (fp16)."""
                e = wpool.tile([128, 2 * PB], F16, name=f"e{l}_{c}",
                               tag=f"e{l}_{c}", bufs=2)
                nc.scalar.activation(e[:], a_cur[:], AF.Exp, bias=negones[:])
                m = wpool.tile([128, 2 * PB], F16, name=f"m{l}_{c}",
                               tag=f"m{l}_{c}", bufs=2)
                h = wpool.tile([128, 2 * PB], F16, name=f"h{l}_{c}",
                               tag=f"h{l}_{c}", bufs=2)
                if h3d_mode:
                    nc.gpsimd.tensor_scalar(out=m[:], in0=e[:], scalar1=1.0,
                                            scalar2=sb["dtc"][:, i:i + 1],
                                            op0=OP.min, op1=OP.mult)
                    nc.vector.scalar_tensor_tensor(out=h[:], in0=a_cur[:],
                                                   scalar=sb["dtc"][:, i:i + 1],
                                                   in1=m[:], op0=OP.mult,
                                                   op1=OP.max)
                else:
                    nc.gpsimd.tensor_scalar(out=m[:], in0=e[:], scalar1=1.0,
                                            scalar2=None, op0=OP.min)
                    nc.vector.tensor_tensor(out=h[:], in0=a_cur[:], in1=m[:],
                                            op=OP.max)
                return h

            def stream_mms(a_next, wkey, h):
                """a_next += W @ h (2 halves x 2 kc), continuing accumulation."""
                for h_ in range(2):
                    for kc in range(2):
                        lh = sb[wkey][:, (kc * 2 + h_) * 128:(kc * 2 + h_ + 1) * 128]
                        nc.tensor.matmul(
                            a_next[:, h_ * PB:(h_ + 1) * PB], lhsT=lh,
                            rhs=h[:, kc * PB:(kc + 1) * PB],
                            start=False, stop=(kc == 1),
                            skip_group_check=True)

            a1_first_mms()

            for i in range(ns):
                # ---- top of step: bias mms (single start per PSUM tile) ----
                if i + 1 < ns:
                    open_a1(i + 1)
                a2 = ppool.tile([128, 2 * G * PB], F32, name="a2",
                                tag="a2", bufs=2)
                a3 = ppool.tile([128, 2 * G * PB], F32, name="a3",
                                tag="a3", bufs=2)
                dxp = ppool.tile([8, G * PB], F32, name="dxp",
                                 tag="dxp", bufs=2)
                nc.tensor.matmul(a2[:], lhsT=sb["b2e"][:],
                                 rhs=sb["bias_ones"][:], start=True,
                                 stop=False, skip_group_check=True)
                nc.tensor.matmul(a3[:], lhsT=sb["b3e"][:],
                                 rhs=sb["bias_ones"][:], start=True,
                                 stop=False, skip_group_check=True)
                # dxp = dt_i * b4' via per-step hilo lhsT slice x ones rhs
                nc.tensor.matmul(dxp[:],
                                 lhsT=sb["db4t"][:, i * 8:(i + 1) * 8],
                                 rhs=sb["ones2"][:],
                                 start=True, stop=False,
                                 skip_group_check=True)

                # ---- layers 1..3 interleaved across chains ----
                h3d = [None] * G
                for c in range(G):
                    h1 = layer(c, i, 1, chs(a1t[i], c), "w2p", False)
                    stream_mms(chs(a2, c), "w2p", h1)
                for c in range(G):
                    h2 = layer(c, i, 2, chs(a2, c), "w3p", False)
                    stream_mms(chs(a3, c), "w3p", h2)
                for c in range(G):
                    h3d[c] = layer(c, i, 3, chs(a3, c), None, True)
                    # W14 + stage into a1_{i+1} (start/stop per region)
                    if i + 1 < ns:
                        a1_tail_mms(c, i + 1, h3d[c])
                    # L4 into dxp
                    for kc in range(2):
                        nc.tensor.matmul(
                            dxp[:, c * PB:(c + 1) * PB],
                            lhsT=sb["w4p"][:, kc * 8:(kc + 1) * 8],
                            rhs=h3d[c][:, kc * PB:(kc + 1) * PB],
                            start=False, stop=(kc == 1),
                            skip_group_check=True)

                # ---- tail: euler, stage writes, ring flush ----
                for c in range(G):
                    g, s = i // RG, i % RG
                    if g not in rings[c]:
                        rings[c] = {}  # drop refs from 2 groups ago
                        rings[c][g] = wpool.tile([8, RG * PB], F32,
                                                 name=f"ring_{c}",
                                                 tag=f"ring_{c}", bufs=2)
                    ring = rings[c][g]
                    xnext = ring[:, s * PB:(s + 1) * PB]
                    xp = xprev[c] if xprev[c] is not None else sb[f"x0f_{c}"][:]
                    nc.vector.tensor_tensor(out=xnext,
                                            in0=dxp[:, c * PB:(c + 1) * PB],
                                            in1=xp, op=OP.add)
                    xprev[c] = xnext
                    if s == RG - 1 or i == ns - 1:
                        nfill = s + 1
                        nc.sync.dma_start(
                            out=xout_d[:, g * RG + 1:g * RG + 1 + nfill,
                                       c * PB:(c + 1) * PB],
                            in_=ring[:, :nfill * PB])
                    # write stage_{i+2}
                    if i + 2 < ns:
                        stn = stages[c][(i + 2) % 4]
                        nc.vector.tensor_scalar(
                            out=stn[0:8, :], in0=xnext,
                            scalar1=sb["db4s"][:, i + 1:i + 2], scalar2=None,
                            op0=OP.add)
                        j = i + 2
                        qb, r = j // 4, j % 4
                        nc.vector.tensor_copy(
                            out=stn[32:40, :],
                            in_=sb[f"zq4_{c}"][32 * r:32 * r + 8,
                                               qb * PB:(qb + 1) * PB])

    nc.compile()
    return nc


_BUILD_CACHE = {}


def _get_compiled(nsteps):
    if nsteps not in _BUILD_CACHE:
        _BUILD_CACHE[nsteps] = _build(nsteps)
    return _BUILD_CACHE[nsteps]


def _hilo(v):
    hi = v.astype(np.float16)
    lo = (v - hi.astype(np.float32)).astype(np.float16)
    return hi, lo


def kernel(t, x, z, event_t, z_jump, W1, b1, W2, b2, W3, b3, W4, b4,
           nsteps=NSTEPS, ng=None):
    global LAST_RESULTS
    t = np.asarray(t, np.float32); x = np.asarray(x, np.float32)
    z = np.asarray(z, np.float32)
    event_t = np.asarray(event_t, np.float32)
    z_jump = np.asarray(z_jump, np.float32)
    W1 = np.asarray(W1, np.float32); b1 = np.asarray(b1, np.float32)
    W2 = np.asarray(W2, np.float32); b2 = np.asarray(b2, np.float32)
    W3 = np.asarray(W3, np.float32); b3 = np.asarray(b3, np.float32)
    W4 = np.asarray(W4, np.float32); b4 = np.asarray(b4, np.float32)

    ns = nsteps
    nzb = (ns + 3) // 4
    nrg = (ns + RG - 1) // RG
    nring = nrg * RG + 1

    tv = t[0, :, 0]
    dt = (tv[1:ns + 1] - tv[:ns]).astype(np.float32)        # [ns]

    # ---- shared weight-derived tensors ----
    W1a, W1b, W1c = W1[0:16], W1[16:32], W1[32:48]
    V = (W1a - W1b).astype(np.float32)
    U = (W1b + W1c).astype(np.float32)
    Ux, Uz = U[:8], U[8:16]
    vux = np.zeros((40, 256), np.float32)
    vux[0:8] = Ux
    vux[8:24] = V
    vux[24], vux[25] = _hilo(b1 + 1.0)
    vux[32:40] = Uz

    def _packw(W):   # [256, 256] -> [128, 512]
        return W.reshape(2, 128, 2, 128).transpose(1, 0, 2, 3).reshape(128, 512)

    W14 = (W4 @ Ux).astype(np.float32)          # [256, 256]
    b4p = (b4 - W4.sum(0)).astype(np.float32)   # b4' [8]

    b2p = (b2 - W2.sum(0) + 1.0).astype(np.float32)
    b3p = (b3 - W3.sum(0) + 1.0).astype(np.float32)
    b2e = np.zeros((4, 128), np.float32)
    b2e[0], b2e[1] = _hilo(b2p[0:128]); b2e[2], b2e[3] = _hilo(b2p[128:256])
    b3e = np.zeros((4, 128), np.float32)
    b3e[0], b3e[1] = _hilo(b3p[0:128]); b3e[2], b3e[3] = _hilo(b3p[128:256])

    bias_ones = np.zeros((4, 2 * G * PB), np.float32)
    for c in range(G):
        bias_ones[0:2, c * 2 * PB: c * 2 * PB + PB] = 1.0
        bias_ones[2:4, c * 2 * PB + PB: (c + 1) * 2 * PB] = 1.0

    dtc = np.broadcast_to(dt, (128, ns)).copy()
    db4s = np.outer(b4p, dt).astype(np.float32)      # [8, ns]
    d4hi, d4lo = _hilo(db4s.T.reshape(-1))           # [ns*8]
    db4t = np.stack([d4hi, d4lo]).astype(np.float16)  # [2, 8*ns]

    shared = dict(
        vux=vux.astype(np.float16),
        w2p=_packw(W2).astype(np.float16),
        w3p=_packw(W3).astype(np.float16),
        w14p=_packw(W14).astype(np.float16),
        w4p=W4.reshape(2, 128, XD).transpose(1, 0, 2).reshape(128, 16).astype(np.float16),
        b2e=b2e.astype(np.float16), b3e=b3e.astype(np.float16),
        db4t=db4t, ones2=np.ones((2, G * PB), np.float16),
        bias_ones=bias_ones.astype(np.float16),
        dtc=dtc, db4s=db4s,
    )

    # ---- per-core, per-chain data ----
    # z_eff on host: z_eff[i] = where(tv[i] >= event_t, z_jump, z[:, i])
    mask = tv[:ns, None] >= event_t[None, :, 0]           # [ns, B]
    in_maps = []
    for core in range(NCORES):
        m = dict(shared)
        for c in range(G):
            bs = slice(core * PCB + c * PB, core * PCB + (c + 1) * PB)
            xb = x[bs]; zb = z[bs]; zjb = z_jump[bs]
            x0 = xb[:, 0]                                  # [PB, 8]
            z0f = zb[:, 0]
            mk = mask[:, bs]                               # [ns, PB]
            zeff = np.where(mk[:, :, None], zjb[None], zb[:, :ns].transpose(1, 0, 2))
            # zeff: [ns, PB, 8] -> zq4 [128, nzb*PB]
            zq4 = np.zeros((128, nzb * PB), np.float16)
            for j in range(ns):
                qb, r = j // 4, j % 4
                zq4[32 * r:32 * r + 8, qb * PB:(qb + 1) * PB] = \
                    zeff[j].T.astype(np.float16)
            def mkstage(xrow, zrow):
                st = np.zeros((40, PB), np.float32)
                st[0:8] = xrow
                st[8:24] = np.concatenate([x0.T, z0f.T], axis=0)
                st[24:26] = 1.0
                st[32:40] = zrow
                return st.astype(np.float16)
            st0 = mkstage(x0.T, zeff[0].T)
            st1 = mkstage(x0.T + db4s[:, 0:1], zeff[min(1, ns - 1)].T)
            m[f"zq4_{c}"] = zq4
            m[f"st0_{c}"] = st0
            m[f"st1_{c}"] = st1
            m[f"x0f_{c}"] = x0.T.astype(np.float32).copy()
        in_maps.append({k: np.ascontiguousarray(v) for k, v in m.items()})

    nc = _get_compiled(ns)
    res = bass_utils.run_bass_kernel_spmd(nc, in_maps,
                                          core_ids=list(range(NCORES)))
    LAST_RESULTS = res

    out = np.zeros((B, T, XD), np.float32)
    n = min(ns + 1, T)
    for core in range(NCORES):
        raw = res.results[core]["xout_d"]          # [8, nring, PCB]
        traj = raw.transpose(2, 1, 0)              # [PCB, nring, 8]
        bs = slice(core * PCB, (core + 1) * PCB)
        out[bs, 0] = x[bs, 0]
        out[bs, 1:n] = traj[:, 1:n]
    return out
